# revision 25
# baseline (speedup 1.0000x reference)
"""Trainium2 Bass kernel for single-head attention (B=8, N=2048, C=512).

Strategy: data-parallel over batch across the 8 NeuronCores — each core
computes one full batch sample. The whole chain is laid out so that NO
on-device transposes are needed:

  per core (b = core id):
    qT[d,n] = (SCALE*w_q) @ x_b^T        (lhsT = w_qT tiles,  rhs = xT)
    kT[d,n] = w_k @ x_b^T                (lhsT = w_kT tiles,  rhs = xT)
    v[m,d]  = x_b @ w_v^T                (lhsT = xT tiles,    rhs = w_vT)
    ST[m,n] = kT^T-tiles @ qT            (= scores transposed, no max-sub)
    PT[m,n] = exp(ST)                    (ACT, PSUM -> SBUF bf16)
    avT[d,n] = sum_m v-tile^T @ PT       (= (P@V)^T, unnormalized)
    s[n]    = ones^T @ (sum_m PT)        (PT summed on DVE, one matmul/chunk)
    yT[e,n] = w_p @ avT                  (unnormalized projection)
  host: out[b] = yT^T / s[:,None] + v + b_proj
  (softmax normalization is linear in the row, so it commutes with the
   projection and is applied on the host)

QKV projections run in float32r (TF32-like, 1 cycle/row, ~2e-4 err);
scores/AV/proj run bf16 (same 1 cycle/row but fast-weight-load halves
the LDWEIGHTS cost; accumulation is always fp32 in PSUM).

Pipelining: xT is loaded in 512-column quarters and QKV is emitted
chunk-outer so the first matmul only waits for ~2MB of DMA; the
projection of chunk ch-1 is emitted after the attention of chunk ch so
its matmuls act as PE filler while attention waits on ACT/DVE.
"""

import ml_dtypes
import numpy as np

import concourse.bass as bass
import concourse.mybir as mybir
import concourse.tile as tile
from concourse import bacc
from concourse.bass_utils import run_bass_kernel_spmd

P = 128           # partitions
N = 2048          # tokens per batch sample
C = 512           # model dim
NT = N // P       # 16 token (m) tiles
CT = C // P       # 4 dim tiles
FB = 512          # free-dim block (n-chunk)
NCH = N // FB     # 4 n-chunks
B = 8             # batch == number of cores
SCALE = C ** -0.5

F32 = mybir.dt.float32
F32R = mybir.dt.float32r
BF16 = mybir.dt.bfloat16
EXP = mybir.ActivationFunctionType.Exp


def build():
    nc = bacc.Bacc("TRN2", target_bir_lowering=False, debug=False)

    xT = nc.dram_tensor("xT", [C, N], F32R, kind="ExternalInput")      # x[b].T
    wqT = nc.dram_tensor("wqT", [C, C], F32R, kind="ExternalInput")    # (SCALE*w_q).T [c,d]
    wkT = nc.dram_tensor("wkT", [C, C], F32R, kind="ExternalInput")    # w_k.T [c,d]
    wvT = nc.dram_tensor("wvT", [C, C], F32R, kind="ExternalInput")    # w_v.T [c,d]
    wpT = nc.dram_tensor("wpT", [C, C], BF16, kind="ExternalInput")     # w_proj.T [d,e]
    yT = nc.dram_tensor("yT", [C, N], F32, kind="ExternalOutput")      # (P@V @ wp.T).T
    sden = nc.dram_tensor("sden", [1, N], F32, kind="ExternalOutput")  # softmax denominators
    vout = nc.dram_tensor("vout", [N, C], F32, kind="ExternalOutput")  # v (for host residual)

    with tile.TileContext(nc) as tc:
        with (
            tc.tile_pool(name="sb", bufs=2) as sb,
            tc.tile_pool(name="ps", bufs=2, space="PSUM") as psp,
        ):
            ones_f32 = sb.tile([P, 1], F32, tag="ones_f32", bufs=1)
            nc.vector.memset(ones_f32, 1.0)
            ones_col = sb.tile([P, 1], F32R, tag="ones", bufs=1)
            nc.vector.tensor_copy(ones_col, ones_f32)

            # ---- input loads, most-urgent first ----
            def load_w(handle, tag, bufs, dtype=F32R):
                ws = []
                for ci in range(CT):
                    t = sb.tile([P, C], dtype, tag=tag, bufs=bufs,
                                name=f"w{handle.name}{ci}")
                    nc.sync.dma_start(t, handle[ci * P:(ci + 1) * P, :])
                    ws.append(t)
                return ws

            def load_xt_chunk(ch, xts):
                for ci in range(CT):
                    t = sb.tile([P, FB], F32R, tag="xt", bufs=16,
                                name=f"xt{ci}_{ch}")
                    nc.sync.dma_start(
                        t, xT[ci * P:(ci + 1) * P, ch * FB:(ch + 1) * FB])
                    xts[(ci, ch)] = t

            # warm the PE clock (HAM) with dummy matmuls while the first
            # DMAs stream in; results are discarded
            warm = sb.tile([P, FB], BF16, tag="warm", bufs=1)
            nc.vector.memset(warm, 0.0)
            pwarm = psp.tile([P, FB], F32, tag="psc", bufs=4, name="pwarm")
            for i in range(14):
                nc.tensor.matmul(pwarm, warm[:, 0:P], warm,
                                 start=True, stop=True)

            # interleave wq and xT-chunk-0 tiles: the first QKV group needs
            # all eight, so issue them round-robin across DMA queues
            xts = {}
            wq = []
            for ci in range(CT):
                t = sb.tile([P, C], F32R, tag="w", bufs=12, name=f"wwqT{ci}")
                nc.sync.dma_start(t, wqT[ci * P:(ci + 1) * P, :])
                wq.append(t)
                t2 = sb.tile([P, FB], F32R, tag="xt", bufs=16,
                             name=f"xt{ci}_0")
                nc.sync.dma_start(t2, xT[ci * P:(ci + 1) * P, 0:FB])
                xts[(ci, 0)] = t2
            wk = load_w(wkT, "w", 12)
            wv = load_w(wvT, "w", 12)
            for ch in range(1, NCH):
                load_xt_chunk(ch, xts)
            wpb = load_w(wpT, "wpb", 4, dtype=BF16)

            # ---- QKV projections, chunk-outer ----
            qts, kts, vs = {}, {}, {}
            for ch in range(NCH):
                for wt, store in ((wq, qts), (wk, kts)):
                    for di in range(CT):
                        ps = psp.tile([P, FB], F32, tag="psc", bufs=4,
                                      name=f"pqk{di}_{ch}")
                        for ci in range(CT):
                            nc.tensor.matmul(
                                ps,
                                wt[ci][:, di * P:(di + 1) * P],
                                xts[(ci, ch)],
                                start=(ci == 0), stop=(ci == CT - 1),
                            )
                        t = sb.tile([P, FB], BF16, tag="qk", bufs=32,
                                    name=f"qk{di}_{ch}")
                        if store is qts:
                            nc.vector.tensor_copy(t, ps)
                        else:
                            nc.scalar.copy(t, ps)
                        store[(di, ch)] = t
                for mi in range(ch * 4, ch * 4 + 4):
                    ps = psp.tile([P, C], F32, tag="pav", bufs=4,
                                  name=f"pv{mi}")
                    for ci in range(CT):
                        nc.tensor.matmul(
                            ps,
                            xts[(ci, ch)][:, (mi % 4) * P:(mi % 4 + 1) * P],
                            wv[ci],
                            start=(ci == 0), stop=(ci == CT - 1),
                        )
                    t = sb.tile([P, C], BF16, tag="v", bufs=16, name=f"v{mi}")
                    nc.vector.tensor_copy(t, ps)
                    vf = sb.tile([P, C], F32, tag="vf", bufs=3, name=f"vf{mi}")
                    nc.scalar.copy(vf, ps)
                    nc.sync.dma_start(vout[mi * P:(mi + 1) * P, :], vf)
                    vs[mi] = t

            # ---- attention per n-chunk; proj(ch-1) emitted after
            # attention(ch) so it fills PE bubbles ----
            saved = {}

            def emit_proj(ch, avts):
                for ei in range(CT):
                    py = psp.tile([P, FB], F32, tag="psc", bufs=4,
                                  name=f"py{ei}_{ch}")
                    for di in range(CT):
                        nc.tensor.matmul(
                            py,
                            wpb[di][:, ei * P:(ei + 1) * P],
                            avts[di],
                            start=(di == 0), stop=(di == CT - 1),
                        )
                    yt = sb.tile([P, FB], F32, tag="yo", bufs=3,
                                 name=f"yt{ei}_{ch}")
                    if ch == NCH - 1 and ei % 2 == 1:
                        nc.scalar.copy(yt, py)
                    else:
                        nc.vector.tensor_copy(yt, py)
                    nc.sync.dma_start(
                        yT[ei * P:(ei + 1) * P, ch * FB:(ch + 1) * FB], yt)

            for ch in range(NCH):
                pavs = [
                    psp.tile([P, FB], F32, tag="pav", bufs=4,
                             name=f"pav{ch}_{di}")
                    for di in range(CT)
                ]
                acc_s = sb.tile([P, FB], F32R, tag="accs", bufs=2,
                                name=f"accs{ch}")
                pts = {}

                def emit_av(mi):
                    pt = pts.pop(mi)
                    for di in range(CT):
                        nc.tensor.matmul(
                            pavs[di],
                            vs[mi][:, di * P:(di + 1) * P],
                            pt,
                            start=(mi == 0), stop=(mi == NT - 1),
                        )

                for mi in range(NT):
                    psc = psp.tile([P, FB], F32, tag="psc", bufs=4,
                                   name=f"psc{ch}_{mi}")
                    for di in range(CT):
                        nc.tensor.matmul(
                            psc,
                            kts[(di, mi // 4)][:, (mi % 4) * P:(mi % 4 + 1) * P],
                            qts[(di, ch)],
                            start=(di == 0), stop=(di == CT - 1),
                        )
                    pt = sb.tile([P, FB], BF16, tag="pt", bufs=16,
                                 name=f"pt{ch}_{mi}")
                    nc.scalar.activation(pt, psc, EXP)
                    if mi == 0:
                        nc.vector.tensor_copy(acc_s, pt)
                    else:
                        nc.vector.tensor_add(acc_s, acc_s, pt)
                    pts[mi] = pt
                    # AV lags two iterations behind: exp(mi-2) had a full
                    # cycle of scores matmuls to complete, so AV never
                    # stalls on ACT latency
                    if mi > 2:
                        emit_av(mi - 3)
                emit_av(NT - 3)
                emit_av(NT - 2)
                emit_av(NT - 1)

                avts = []
                for di in range(CT):
                    t = sb.tile([P, FB], BF16, tag="avt", bufs=16,
                                name=f"avt{ch}_{di}")
                    if ch == NCH - 1 and di % 2 == 1:
                        nc.scalar.copy(t, pavs[di])
                    else:
                        nc.vector.tensor_copy(t, pavs[di])
                    avts.append(t)
                saved[ch] = avts

                ps_s = psp.tile([1, FB], F32, tag="psc", bufs=4,
                                name=f"ps_s{ch}")
                nc.tensor.matmul(ps_s, ones_col, acc_s, start=True, stop=True)
                s_sb = sb.tile([1, FB], F32, tag="s", bufs=4, name=f"s{ch}")
                nc.vector.tensor_copy(s_sb, ps_s)
                nc.sync.dma_start(sden[:, ch * FB:(ch + 1) * FB], s_sb)

                if ch > 0:
                    emit_proj(ch - 1, saved.pop(ch - 1))
            emit_proj(NCH - 1, saved.pop(NCH - 1))

    nc.compile()
    return nc


_NC = None


def _get_nc():
    global _NC
    if _NC is None:
        _NC = build()
    return _NC


def kernel(x, w_qkv, w_proj, b_proj):
    x = np.asarray(x, dtype=np.float32)
    w_qkv = np.asarray(w_qkv, dtype=np.float32)
    w_proj = np.asarray(w_proj, dtype=np.float32)
    b_proj = np.asarray(b_proj, dtype=np.float32)

    bf16 = ml_dtypes.bfloat16
    wq = np.ascontiguousarray((w_qkv[0:C] * SCALE).T)
    wk = np.ascontiguousarray(w_qkv[C:2 * C].T)
    wv = np.ascontiguousarray(w_qkv[2 * C:3 * C].T)
    wp = np.ascontiguousarray(w_proj.T.astype(bf16))

    in_maps = []
    for b in range(B):
        in_maps.append({
            "xT": np.ascontiguousarray(x[b].T),
            "wqT": wq, "wkT": wk, "wvT": wv, "wpT": wp,
        })

    nc = _get_nc()
    res = None
    for attempt in range(3):
        try:
            res = run_bass_kernel_spmd(nc, in_maps, core_ids=list(range(B)))
            break
        except Exception:
            if attempt == 2:
                raise
            import time
            time.sleep(5)

    out = np.empty((B, N, C), np.float32)
    for b in range(B):
        r = res.results[b]
        s = r["sden"].reshape(N, 1)
        out[b] = r["yT"].T / s + r["vout"] + b_proj[None, :]
    return out


# revision 26
# speedup vs baseline: 1.0090x; 1.0090x over previous
"""Trainium2 Bass kernel for single-head attention (B=8, N=2048, C=512).

Strategy: data-parallel over batch across the 8 NeuronCores — each core
computes one full batch sample. The whole chain is laid out so that NO
on-device transposes are needed:

  per core (b = core id):
    qT[d,n] = (SCALE*w_q) @ x_b^T        (lhsT = w_qT tiles,  rhs = xT)
    kT[d,n] = w_k @ x_b^T                (lhsT = w_kT tiles,  rhs = xT)
    v[m,d]  = x_b @ w_v^T                (lhsT = xT tiles,    rhs = w_vT)
    ST[m,n] = kT^T-tiles @ qT            (= scores transposed, no max-sub)
    PT[m,n] = exp(ST)                    (ACT, PSUM -> SBUF bf16)
    avT[d,n] = sum_m v-tile^T @ PT       (= (P@V)^T, unnormalized)
    s[n]    = ones^T @ (sum_m PT)        (PT summed on DVE, one matmul/chunk)
    yT[e,n] = w_p @ avT                  (unnormalized projection)
  host: out[b] = yT^T / s[:,None] + v + b_proj
  (softmax normalization is linear in the row, so it commutes with the
   projection and is applied on the host)

QKV projections run in float32r (TF32-like, 1 cycle/row, ~2e-4 err);
scores/AV/proj run bf16 (same 1 cycle/row but fast-weight-load halves
the LDWEIGHTS cost; accumulation is always fp32 in PSUM).

Pipelining: xT is loaded in 512-column quarters and QKV is emitted
chunk-outer so the first matmul only waits for ~2MB of DMA; the
projection of chunk ch-1 is emitted after the attention of chunk ch so
its matmuls act as PE filler while attention waits on ACT/DVE.
"""

import ml_dtypes
import numpy as np

import concourse.bass as bass
import concourse.mybir as mybir
import concourse.tile as tile
from concourse import bacc
from concourse.bass_utils import run_bass_kernel_spmd

P = 128           # partitions
N = 2048          # tokens per batch sample
C = 512           # model dim
NT = N // P       # 16 token (m) tiles
CT = C // P       # 4 dim tiles
FB = 512          # free-dim block (n-chunk)
NCH = N // FB     # 4 n-chunks
B = 8             # batch == number of cores
SCALE = C ** -0.5

F32 = mybir.dt.float32
F32R = mybir.dt.float32r
BF16 = mybir.dt.bfloat16
EXP = mybir.ActivationFunctionType.Exp


def build():
    nc = bacc.Bacc("TRN2", target_bir_lowering=False, debug=False)

    xT = nc.dram_tensor("xT", [C, N], F32R, kind="ExternalInput")      # x[b].T
    wqT = nc.dram_tensor("wqT", [C, C], F32R, kind="ExternalInput")    # (SCALE*w_q).T [c,d]
    wkT = nc.dram_tensor("wkT", [C, C], F32R, kind="ExternalInput")    # w_k.T [c,d]
    wvT = nc.dram_tensor("wvT", [C, C], F32R, kind="ExternalInput")    # w_v.T [c,d]
    wpT = nc.dram_tensor("wpT", [C, C], BF16, kind="ExternalInput")     # w_proj.T [d,e]
    yT = nc.dram_tensor("yT", [C, N], F32, kind="ExternalOutput")      # (P@V @ wp.T).T
    sden = nc.dram_tensor("sden", [1, N], F32, kind="ExternalOutput")  # softmax denominators
    vout = nc.dram_tensor("vout", [N, C], F32, kind="ExternalOutput")  # v (for host residual)

    with tile.TileContext(nc) as tc:
        with (
            tc.tile_pool(name="sb", bufs=2) as sb,
            tc.tile_pool(name="ps", bufs=2, space="PSUM") as psp,
        ):
            ones_f32 = sb.tile([P, 1], F32, tag="ones_f32", bufs=1)
            nc.vector.memset(ones_f32, 1.0)
            ones_col = sb.tile([P, 1], F32R, tag="ones", bufs=1)
            nc.vector.tensor_copy(ones_col, ones_f32)

            # ---- input loads, most-urgent first ----
            def load_w(handle, tag, bufs, dtype=F32R):
                ws = []
                for ci in range(CT):
                    t = sb.tile([P, C], dtype, tag=tag, bufs=bufs,
                                name=f"w{handle.name}{ci}")
                    nc.sync.dma_start(t, handle[ci * P:(ci + 1) * P, :])
                    ws.append(t)
                return ws

            def load_xt_chunk(ch, xts):
                for ci in range(CT):
                    t = sb.tile([P, FB], F32R, tag="xt", bufs=16,
                                name=f"xt{ci}_{ch}")
                    nc.sync.dma_start(
                        t, xT[ci * P:(ci + 1) * P, ch * FB:(ch + 1) * FB])
                    xts[(ci, ch)] = t

            # warm the PE clock (HAM) with dummy matmuls while the first
            # DMAs stream in; results are discarded
            warm = sb.tile([P, FB], BF16, tag="warm", bufs=1)
            nc.vector.memset(warm, 0.0)
            pwarm = psp.tile([P, FB], F32, tag="psc", bufs=4, name="pwarm")
            for i in range(14):
                nc.tensor.matmul(pwarm, warm[:, 0:P], warm,
                                 start=True, stop=True)

            # interleave wq and xT-chunk-0 tiles: the first QKV group needs
            # all eight, so issue them round-robin across DMA queues
            xts = {}
            wq = []
            for ci in range(CT):
                t = sb.tile([P, C], F32R, tag="w", bufs=12, name=f"wwqT{ci}")
                nc.sync.dma_start(t, wqT[ci * P:(ci + 1) * P, :])
                wq.append(t)
                t2 = sb.tile([P, FB], F32R, tag="xt", bufs=16,
                             name=f"xt{ci}_0")
                nc.sync.dma_start(t2, xT[ci * P:(ci + 1) * P, 0:FB])
                xts[(ci, 0)] = t2
            wk = load_w(wkT, "w", 12)
            wv = load_w(wvT, "w", 12)
            for ch in range(1, NCH):
                load_xt_chunk(ch, xts)
            wpb = load_w(wpT, "wpb", 4, dtype=BF16)

            # ---- QKV projections, chunk-outer ----
            qts, kts, vs = {}, {}, {}
            for ch in range(NCH):
                for wt, store in ((wq, qts), (wk, kts)):
                    for di in range(CT):
                        ps = psp.tile([P, FB], F32, tag="psc", bufs=4,
                                      name=f"pqk{di}_{ch}")
                        for ci in range(CT):
                            nc.tensor.matmul(
                                ps,
                                wt[ci][:, di * P:(di + 1) * P],
                                xts[(ci, ch)],
                                start=(ci == 0), stop=(ci == CT - 1),
                            )
                        t = sb.tile([P, FB], BF16, tag="qk", bufs=32,
                                    name=f"qk{di}_{ch}")
                        if store is qts:
                            nc.vector.tensor_copy(t, ps)
                        else:
                            nc.scalar.copy(t, ps)
                        store[(di, ch)] = t
                for mi in range(ch * 4, ch * 4 + 4):
                    ps = psp.tile([P, C], F32, tag="pav", bufs=4,
                                  name=f"pv{mi}")
                    for ci in range(CT):
                        nc.tensor.matmul(
                            ps,
                            xts[(ci, ch)][:, (mi % 4) * P:(mi % 4 + 1) * P],
                            wv[ci],
                            start=(ci == 0), stop=(ci == CT - 1),
                        )
                    t = sb.tile([P, C], BF16, tag="v", bufs=16, name=f"v{mi}")
                    nc.vector.tensor_copy(t, ps)
                    vf = sb.tile([P, C], F32, tag="vf", bufs=3, name=f"vf{mi}")
                    nc.scalar.copy(vf, ps)
                    nc.sync.dma_start(vout[mi * P:(mi + 1) * P, :], vf)
                    vs[mi] = t

            # ---- attention per n-chunk; proj(ch-1) emitted after
            # attention(ch) so it fills PE bubbles ----
            saved = {}

            def emit_proj(ch, avts):
                for ei in range(CT):
                    py = psp.tile([P, FB], F32, tag="psc", bufs=4,
                                  name=f"py{ei}_{ch}")
                    for di in range(CT):
                        nc.tensor.matmul(
                            py,
                            wpb[di][:, ei * P:(ei + 1) * P],
                            avts[di],
                            start=(di == 0), stop=(di == CT - 1),
                        )
                    yt = sb.tile([P, FB], F32, tag="yo", bufs=3,
                                 name=f"yt{ei}_{ch}")
                    if ch == NCH - 1 and ei % 2 == 1:
                        nc.scalar.copy(yt, py)
                    else:
                        nc.vector.tensor_copy(yt, py)
                    nc.sync.dma_start(
                        yT[ei * P:(ei + 1) * P, ch * FB:(ch + 1) * FB], yt)

            for ch in range(NCH):
                pavs = [
                    psp.tile([P, FB], F32, tag="pav", bufs=4,
                             name=f"pav{ch}_{di}")
                    for di in range(CT)
                ]
                acc_s = sb.tile([P, FB], F32R, tag="accs", bufs=2,
                                name=f"accs{ch}")
                pts = {}

                def emit_av(mi):
                    pt = pts.pop(mi)
                    for di in range(CT):
                        nc.tensor.matmul(
                            pavs[di],
                            vs[mi][:, di * P:(di + 1) * P],
                            pt,
                            start=(mi == 0), stop=(mi == NT - 1),
                        )

                for mi in range(NT):
                    psc = psp.tile([P, FB], F32, tag="psc", bufs=4,
                                   name=f"psc{ch}_{mi}")
                    for di in range(CT):
                        nc.tensor.matmul(
                            psc,
                            kts[(di, mi // 4)][:, (mi % 4) * P:(mi % 4 + 1) * P],
                            qts[(di, ch)],
                            start=(di == 0), stop=(di == CT - 1),
                        )
                    pt = sb.tile([P, FB], BF16, tag="pt", bufs=16,
                                 name=f"pt{ch}_{mi}")
                    nc.scalar.activation(pt, psc, EXP)
                    if mi == 0:
                        nc.vector.tensor_copy(acc_s, pt)
                    else:
                        nc.vector.tensor_add(acc_s, acc_s, pt)
                    pts[mi] = pt
                    # AV lags two iterations behind: exp(mi-2) had a full
                    # cycle of scores matmuls to complete, so AV never
                    # stalls on ACT latency
                    if mi > 1:
                        emit_av(mi - 2)
                emit_av(NT - 2)
                emit_av(NT - 1)

                avts = []
                for di in range(CT):
                    t = sb.tile([P, FB], BF16, tag="avt", bufs=16,
                                name=f"avt{ch}_{di}")
                    if ch == NCH - 1 and di % 2 == 1:
                        nc.scalar.copy(t, pavs[di])
                    else:
                        nc.vector.tensor_copy(t, pavs[di])
                    avts.append(t)
                saved[ch] = avts

                ps_s = psp.tile([1, FB], F32, tag="psc", bufs=4,
                                name=f"ps_s{ch}")
                nc.tensor.matmul(ps_s, ones_col, acc_s, start=True, stop=True)
                s_sb = sb.tile([1, FB], F32, tag="s", bufs=4, name=f"s{ch}")
                nc.vector.tensor_copy(s_sb, ps_s)
                nc.sync.dma_start(sden[:, ch * FB:(ch + 1) * FB], s_sb)

                if ch > 0:
                    emit_proj(ch - 1, saved.pop(ch - 1))
            emit_proj(NCH - 1, saved.pop(NCH - 1))

    nc.compile()
    return nc


_NC = None


def _get_nc():
    global _NC
    if _NC is None:
        _NC = build()
    return _NC


def kernel(x, w_qkv, w_proj, b_proj):
    x = np.asarray(x, dtype=np.float32)
    w_qkv = np.asarray(w_qkv, dtype=np.float32)
    w_proj = np.asarray(w_proj, dtype=np.float32)
    b_proj = np.asarray(b_proj, dtype=np.float32)

    bf16 = ml_dtypes.bfloat16
    wq = np.ascontiguousarray((w_qkv[0:C] * SCALE).T)
    wk = np.ascontiguousarray(w_qkv[C:2 * C].T)
    wv = np.ascontiguousarray(w_qkv[2 * C:3 * C].T)
    wp = np.ascontiguousarray(w_proj.T.astype(bf16))

    in_maps = []
    for b in range(B):
        in_maps.append({
            "xT": np.ascontiguousarray(x[b].T),
            "wqT": wq, "wkT": wk, "wvT": wv, "wpT": wp,
        })

    nc = _get_nc()
    res = None
    for attempt in range(3):
        try:
            res = run_bass_kernel_spmd(nc, in_maps, core_ids=list(range(B)))
            break
        except Exception:
            if attempt == 2:
                raise
            import time
            time.sleep(5)

    out = np.empty((B, N, C), np.float32)
    for b in range(B):
        r = res.results[b]
        s = r["sden"].reshape(N, 1)
        out[b] = r["yT"].T / s + r["vout"] + b_proj[None, :]
    return out
